# revision 22
# baseline (speedup 1.0000x reference)
"""Trainium2 Bass kernel for nn_ConvAttention_34600256537137.

Math notes (validated against the reference):
  qkv = 1x1conv(x, w1)+b1 -> Q,K,V;  score = conv5x5(Q_s)+conv5x5(K_t)+b2;
  attn = softmax_t(score);  out = einsum(attn, V).
  Softmax over t is shift-invariant, so the Q-half of the score (constant in
  t), b2, and the K-path bias all cancel.  The computation collapses to:
    weff[ci,dy,dx] = sum_c w1K[c,ci] * w2K[c,dy,dx]        (host, tiny)
    sK[b,t,h,w]    = conv5x5_reflect(x[b,:,:,:,t], weff)
    e = exp(sK);  den = sum_t e
    out[b,o,h,w,s] = (sum_{ci,t} w1V[o,ci] * e * x) / den + b1V[o]
  (s-independent; normalization folded to the end; bias + S-broadcast on host)

Sharding: 8 cores = (b in {0,1}) x (4 chunks of 8 rows of H).

Perf structure (v5; bf16 datapath, rel err ~4e-3 vs 2e-2 budget):
  - conv contraction folded over (ci, t%2): K=128, M=50 (tap x t-parity with
    zero padding), so 12 row-matmuls of 288 cols replace 24 and PSUM->SBUF
    copies run on 50 lanes instead of 25.
  - T lands in DRAM dy-pre-shifted (8-row windows) so (t2,h) merges into one
    stride-36 dim and each (dy, tp) gather is a legal 3-dim DMA: 5 writes +
    10 gathers instead of 25; DVE reduces the 25 taps on 128 lanes.
  - softmax denominator via indicator-matmul on PE; e replicated to the
    (ci8,t)-partition layout with two parallel 0-stride-AP DMAs.
  - xattn split across vector/gpsimd; V matmuls pipeline behind it.
  - device emits only the s-independent [C, HW] tile; the S broadcast and
    +b1V happen on host.
"""

import sys

if "/opt/trn_rl_repo" not in sys.path:
    sys.path.insert(0, "/opt/trn_rl_repo")

import numpy as np
import ml_dtypes

BF16 = ml_dtypes.bfloat16

B, C, H, W, S = 2, 64, 32, 32, 16
KS, PAD = 5, 2
NCORES = 8
ROWS = H // 4            # output rows per core
SLAB_R = ROWS + 2 * PAD  # 12
SLAB_W = W + 2 * PAD     # 36
NTAP = KS * KS           # 25
HW = ROWS * W            # 256
S2 = S // 2              # 8 frame-pairs
M2 = 2 * NTAP            # 50 stationary columns (tap, t-parity)
TAPW = S2 * ROWS * SLAB_W  # 2304: td elements per (tap, tp) plane

_MODULE = None


def _build_module():
    import concourse.bacc as bacc
    import concourse.bass as bass
    import concourse.tile as tile
    from concourse import mybir

    f32 = mybir.dt.float32
    bf16 = mybir.dt.bfloat16
    AF = mybir.ActivationFunctionType
    ALU = mybir.AluOpType
    nc = bacc.Bacc("TRN2", target_bir_lowering=False, debug=False, num_devices=NCORES)

    # slab partitions are (ci, t%2); innermost frame axis is t2 = t//2
    slab_d = nc.dram_tensor("slab", [128, SLAB_R, SLAB_W, S2], bf16, kind="ExternalInput")
    xt_d = nc.dram_tensor("xt", [128, 8, HW], bf16, kind="ExternalInput")
    weff_d = nc.dram_tensor("weff", [128, M2], bf16, kind="ExternalInput")
    w1vr_d = nc.dram_tensor("w1vr", [128, 8, C], bf16, kind="ExternalInput")
    hsel_d = nc.dram_tensor("hsel", [128, ROWS], bf16, kind="ExternalInput")
    o_d = nc.dram_tensor("o", [C, HW], f32, kind="ExternalOutput")
    dn_d = nc.dram_tensor("dn", [ROWS * W], f32, kind="ExternalOutput")

    # scratch DRAM for partition-crossing rearrangements.
    # td holds T with rows pre-shifted by each tap's dy (8-row windows), so
    # (t2,h) merges into a single stride-36 dim and each (dy,tp) gather is a
    # legal 3-dim DMA pattern.
    td_d = nc.dram_tensor("td", [M2, S2, ROWS, SLAB_W], bf16)
    ed_d = nc.dram_tensor("ed", [S, ROWS, W], bf16)               # exp(sK), frame-major

    with tile.TileContext(nc) as tc:
        with tc.tile_pool(name="sb", bufs=1) as sb, tc.tile_pool(
            name="ps", bufs=6, space="PSUM"
        ) as ps, tc.tile_pool(name="pso", bufs=1, space="PSUM") as pso:
            # --- loads: slab split as single rows up front (fast first
            # matmul) then two-row chunks round-robin.  xt/w1vr are deferred
            # to gpsimd's idle mid-kernel window so the slab and T traffic
            # own clean queues. ---
            s_weff = sb.tile([128, M2], bf16)
            nc.gpsimd.dma_start(s_weff, weff_d.ap())
            s_hsel = sb.tile([128, ROWS], bf16)
            nc.gpsimd.dma_start(s_hsel, hsel_d.ap())
            s_slab = sb.tile([128, SLAB_R, SLAB_W, S2], bf16)
            chunks = (
                (0, 1, nc.sync), (1, 2, nc.scalar), (2, 4, nc.gpsimd),
                (4, 6, nc.sync), (6, 8, nc.scalar), (8, 10, nc.gpsimd),
                (10, 12, nc.sync),
            )
            for r0, r1, e in chunks:
                e.dma_start(
                    s_slab[:, r0:r1, :, :], slab_d.ap()[:, r0:r1, :, :]
                )
            # xt/w1vr tiles are loaded later, between the T writebacks, so
            # their bulk transfers don't steal DMA-pool bandwidth from the
            # slab chunks that feed the conv matmuls
            s_xt = sb.tile([128, 8, HW], bf16)
            s_w1vr = sb.tile([128, 8, C], bf16)

            # --- phase A: T[(tap,tp), (w,t2)] = weff^T @ slab, one matmul per
            # slab row; copies transpose to t2-major (tap', t2, row, w) ---
            s_T = sb.tile([M2, S2, SLAB_R, SLAB_W], bf16)
            for row in range(SLAB_R):
                p_t = ps.tile([M2, SLAB_W, S2], f32, tag="pt")
                nc.tensor.matmul(
                    p_t, s_weff, s_slab[:, row, :, :], start=True, stop=True
                )
                dst = s_T[:, :, row, :]
                if row % 2 == 0:
                    nc.scalar.copy(dst, p_t.transpose([0, 2, 1]))
                else:
                    nc.vector.tensor_copy(dst, p_t.transpose([0, 2, 1]))

            # --- T to DRAM: 5 dy-class writes of dy-shifted 8-row windows,
            # each followed by its two (tp) 3-dim batched gathers into
            # R[(tp,t2,h), (dy,dx), w].  Write dy depends on copies for rows
            # dy..dy+7 only, so early dy classes pipeline behind the conv. ---
            s_R = sb.tile([128, NTAP, W], bf16)
            g_engs = (nc.sync, nc.gpsimd, nc.scalar, nc.sync, nc.scalar)
            for dy in range(KS):
                e = g_engs[dy]
                e.dma_start(
                    td_d.ap()[2 * KS * dy : 2 * KS * dy + 2 * KS],
                    s_T[2 * KS * dy : 2 * KS * dy + 2 * KS, :, dy : dy + ROWS, :],
                )
                for tp in range(2):
                    src = bass.AP(
                        tensor=td_d.ap().tensor,
                        offset=2 * KS * TAPW * dy + TAPW * tp,
                        ap=[[SLAB_W, S2 * ROWS], [2 * TAPW + 1, KS], [1, W]],
                    )
                    e.dma_start(
                        s_R[64 * tp : 64 * tp + 64, KS * dy : KS * dy + KS, :], src
                    )
                if dy == 1:
                    # gpsimd's queue is idle from here to the softmax: stream
                    # the V-path operands on it without delaying the
                    # dy2..dy4 writes/gathers on sync/scalar
                    nc.gpsimd.dma_start(s_w1vr, w1vr_d.ap())
                    for q in range(4):
                        nc.gpsimd.dma_start(
                            s_xt[:, 2 * q : 2 * q + 2, :],
                            xt_d.ap()[:, 2 * q : 2 * q + 2, :],
                        )



            # --- tap reduce as a contiguous pairwise tree (strided
            # tensor_reduce over the tap axis runs ~3.5x below DVE peak) ---
            s_t1 = sb.tile([128, 12, W], f32)
            nc.vector.tensor_tensor(
                s_t1, s_R[:, 0:12, :], s_R[:, 12:24, :], op=ALU.add
            )
            s_t2 = sb.tile([128, 6, W], f32)
            nc.vector.tensor_tensor(
                s_t2, s_t1[:, 0:6, :], s_t1[:, 6:12, :], op=ALU.add
            )
            s_t3 = sb.tile([128, 3, W], f32)
            nc.vector.tensor_tensor(
                s_t3, s_t2[:, 0:3, :], s_t2[:, 3:6, :], op=ALU.add
            )
            s_t4 = sb.tile([128, W], f32)
            nc.vector.tensor_tensor(
                s_t4, s_t3[:, 0, :], s_t3[:, 1, :], op=ALU.add
            )
            s_t5 = sb.tile([128, W], f32)
            nc.vector.tensor_tensor(s_t5, s_t3[:, 2, :], s_R[:, 24, :], op=ALU.add)
            s_sk = sb.tile([128, W], f32)  # [(tp,t2,h), w]
            nc.vector.tensor_tensor(s_sk, s_t4, s_t5, op=ALU.add)

            # --- e = exp(sK); den via indicator-matmul on PE ---
            s_e16 = sb.tile([128, W], bf16)
            nc.scalar.activation(s_e16, s_sk, AF.Exp)
            p_den = pso.tile([ROWS, W], f32, tag="den")
            nc.tensor.matmul(p_den, s_hsel, s_e16, start=True, stop=True)
            s_rcp = sb.tile([ROWS, W], f32)
            nc.vector.reciprocal(s_rcp, p_den)
            nc.gpsimd.dma_start(dn_d.ap(), s_rcp)

            # --- bounce e to frame-major [t, hw] (2 writes, one per parity);
            # replicate to [(ci8,t), hw] with two parallel 0-stride reads ---
            for tp, e in ((0, nc.scalar), (1, nc.sync)):
                e.dma_start(
                    bass.AP(
                        tensor=ed_d.ap().tensor,
                        offset=HW * tp,
                        ap=[[2 * HW, S2], [W, ROWS], [1, W]],
                    ),
                    s_e16[64 * tp : 64 * tp + 64, :],
                )
            s_eb = sb.tile([128, HW], bf16)
            for half, e in ((0, nc.scalar), (1, nc.sync)):
                e.dma_start(
                    s_eb[64 * half : 64 * half + 64, :],
                    bass.AP(
                        tensor=ed_d.ap().tensor,
                        offset=0,
                        ap=[[0, 4], [HW, S], [1, HW]],
                    ),
                )

            # --- V path: xattn = x_t * e in two chunks so the first V
            # matmuls overlap the second multiply ---
            s_xa = sb.tile([128, 8, HW], bf16)
            ebb = s_eb.unsqueeze(1).broadcast_to((128, 4, HW))
            nc.vector.tensor_tensor(s_xa[:, 0:4, :], s_xt[:, 0:4, :], ebb, op=ALU.mult)
            nc.vector.tensor_tensor(s_xa[:, 4:8, :], s_xt[:, 4:8, :], ebb, op=ALU.mult)
            p_o = pso.tile([C, HW], f32, tag="out")
            for g in range(8):
                nc.tensor.matmul(
                    p_o,
                    s_w1vr[:, g, :],
                    s_xa[:, g, :],
                    start=(g == 0),
                    stop=(g == 7),
                )
            # raw (unnormalized) output; the host divides by den via dn
            s_o = sb.tile([C, HW], f32)
            nc.vector.tensor_copy(s_o, p_o)
            nc.sync.dma_start(o_d.ap(), s_o)

    nc.compile()
    return nc


def _get_module():
    global _MODULE
    if _MODULE is None:
        _MODULE = _build_module()
    return _MODULE


def make_host_inputs(x, w1, b1, w2, b2):
    """Host-side precompute: folded weights + per-core reflect-padded slices."""
    x = np.ascontiguousarray(np.asarray(x, np.float32))
    w1 = np.asarray(w1, np.float32)
    w2 = np.asarray(w2, np.float32)

    w1K = w1[C : 2 * C, :, 0, 0]          # [c, ci]
    w2K = w2[0, C : 2 * C]                # [c, 5, 5]
    weff = np.einsum("ci,cyx->iyx", w1K, w2K).reshape(C, NTAP)
    # weff2[(ci,tp), (tap,tpo)] = weff[ci, tap] * [tpo == tp]
    weff2 = np.zeros((128, M2), np.float32)
    for tp in range(2):
        weff2[tp::2, tp::2] = weff
    weff2 = weff2.astype(BF16)
    w1V = w1[2 * C :, :, 0, 0]            # [co, ci]

    # w1vr[(ci8,t), g, co] = w1V[co, 8g+ci8]
    tmp = w1V.T.reshape(8, 8, C)                      # (g, ci8, co)
    w1vr = np.ascontiguousarray(
        np.broadcast_to(tmp[:, :, None, :], (8, 8, S, C))
        .transpose(1, 2, 0, 3)
        .reshape(128, 8, C)
    ).astype(BF16)

    # hsel[(tp,t2,h), m] = 1 if h == m
    hsel = np.zeros((128, ROWS), np.float32)
    for p in range(128):
        hsel[p, p % ROWS] = 1.0
    hsel = hsel.astype(BF16)

    in_maps = []
    for core in range(NCORES):
        b, hc = divmod(core, 4)
        h0 = ROWS * hc
        xp = np.pad(x[b], ((0, 0), (PAD, PAD), (PAD, PAD), (0, 0)), mode="reflect")
        sl = xp[:, h0 : h0 + SLAB_R, :, :]            # [ci, r, w36, t]
        slab = np.ascontiguousarray(
            sl.reshape(C, SLAB_R, SLAB_W, S2, 2)
            .transpose(0, 4, 1, 2, 3)
            .reshape(128, SLAB_R, SLAB_W, S2)
        ).astype(BF16)
        xs = x[b][:, h0 : h0 + ROWS, :, :]            # [ci, h, w, t]
        xt = np.ascontiguousarray(
            xs.reshape(8, 8, ROWS, W, S)
            .transpose(1, 4, 0, 2, 3)
            .reshape(128, 8, HW)
        ).astype(BF16)
        in_maps.append(
            {"slab": slab, "xt": xt, "weff": weff2, "w1vr": w1vr, "hsel": hsel}
        )
    return in_maps


def assemble_output(results, b1):
    b1V = np.asarray(b1, np.float32)[2 * C :]
    out = np.empty((B, C, H, W, S), np.float32)
    for core in range(NCORES):
        b, hc = divmod(core, 4)
        h0 = ROWS * hc
        r = results[core]
        o = r["o"].reshape(C, ROWS, W) * r["dn"].reshape(1, ROWS, W)
        out[b, :, h0 : h0 + ROWS, :, :] = o[:, :, :, None]
    out += b1V[None, :, None, None, None]
    return out


def kernel(x, w1, b1, w2, b2):
    from concourse.bass_utils import run_bass_kernel_spmd

    nc = _get_module()
    in_maps = make_host_inputs(x, w1, b1, w2, b2)
    res = run_bass_kernel_spmd(nc, in_maps, core_ids=list(range(NCORES)))
    return assemble_output(res.results, b1)


# revision 24
# speedup vs baseline: 1.0209x; 1.0209x over previous
"""Trainium2 Bass kernel for nn_ConvAttention_34600256537137.

Math notes (validated against the reference):
  qkv = 1x1conv(x, w1)+b1 -> Q,K,V;  score = conv5x5(Q_s)+conv5x5(K_t)+b2;
  attn = softmax_t(score);  out = einsum(attn, V).
  Softmax over t is shift-invariant, so the Q-half of the score (constant in
  t), b2, and the K-path bias all cancel.  The computation collapses to:
    weff[ci,dy,dx] = sum_c w1K[c,ci] * w2K[c,dy,dx]        (host, tiny)
    sK[b,t,h,w]    = conv5x5_reflect(x[b,:,:,:,t], weff)
    e = exp(sK);  den = sum_t e
    out[b,o,h,w,s] = (sum_{ci,t} w1V[o,ci] * e * x) / den + b1V[o]
  (s-independent; normalization folded to the end; bias + S-broadcast on host)

Sharding: 8 cores = (b in {0,1}) x (4 chunks of 8 rows of H).

Perf structure (v5; bf16 datapath, rel err ~4e-3 vs 2e-2 budget):
  - conv contraction folded over (ci, t%2): K=128, M=50 (tap x t-parity with
    zero padding), so 12 row-matmuls of 288 cols replace 24 and PSUM->SBUF
    copies run on 50 lanes instead of 25.
  - T lands in DRAM dy-pre-shifted (8-row windows) so (t2,h) merges into one
    stride-36 dim and each (dy, tp) gather is a legal 3-dim DMA: 5 writes +
    10 gathers instead of 25; DVE reduces the 25 taps on 128 lanes.
  - softmax denominator via indicator-matmul on PE; e replicated to the
    (ci8,t)-partition layout with two parallel 0-stride-AP DMAs.
  - xattn split across vector/gpsimd; V matmuls pipeline behind it.
  - device emits only the s-independent [C, HW] tile; the S broadcast and
    +b1V happen on host.
"""

import sys

if "/opt/trn_rl_repo" not in sys.path:
    sys.path.insert(0, "/opt/trn_rl_repo")

import numpy as np
import ml_dtypes

BF16 = ml_dtypes.bfloat16

B, C, H, W, S = 2, 64, 32, 32, 16
KS, PAD = 5, 2
NCORES = 8
ROWS = H // 4            # output rows per core
SLAB_R = ROWS + 2 * PAD  # 12
SLAB_W = W + 2 * PAD     # 36
NTAP = KS * KS           # 25
HW = ROWS * W            # 256
S2 = S // 2              # 8 frame-pairs
M2 = 2 * NTAP            # 50 stationary columns (tap, t-parity)
TAPW = S2 * ROWS * SLAB_W  # 2304: td elements per (tap, tp) plane

_MODULE = None


def _build_module():
    import concourse.bacc as bacc
    import concourse.bass as bass
    import concourse.tile as tile
    from concourse import mybir

    f32 = mybir.dt.float32
    bf16 = mybir.dt.bfloat16
    AF = mybir.ActivationFunctionType
    ALU = mybir.AluOpType
    nc = bacc.Bacc("TRN2", target_bir_lowering=False, debug=False, num_devices=NCORES)

    # slab partitions are (ci, t%2); innermost frame axis is t2 = t//2
    slab_d = nc.dram_tensor("slab", [128, SLAB_R, SLAB_W, S2], bf16, kind="ExternalInput")
    xt_d = nc.dram_tensor("xt", [128, 8, HW], bf16, kind="ExternalInput")
    weff_d = nc.dram_tensor("weff", [128, M2], bf16, kind="ExternalInput")
    w1vr_d = nc.dram_tensor("w1vr", [128, 8, C], bf16, kind="ExternalInput")
    hsel_d = nc.dram_tensor("hsel", [128, ROWS], bf16, kind="ExternalInput")
    o_d = nc.dram_tensor("o", [C, HW], f32, kind="ExternalOutput")
    dn_d = nc.dram_tensor("dn", [ROWS * W], f32, kind="ExternalOutput")

    # scratch DRAM for partition-crossing rearrangements.
    # td holds T with rows pre-shifted by each tap's dy (8-row windows), so
    # (t2,h) merges into a single stride-36 dim and each (dy,tp) gather is a
    # legal 3-dim DMA pattern.
    td_d = nc.dram_tensor("td", [M2, S2, ROWS, SLAB_W], bf16)
    ed_d = nc.dram_tensor("ed", [S, ROWS, W], bf16)               # exp(sK), frame-major

    with tile.TileContext(nc) as tc:
        with tc.tile_pool(name="sb", bufs=1) as sb, tc.tile_pool(
            name="ps", bufs=6, space="PSUM"
        ) as ps, tc.tile_pool(name="pso", bufs=1, space="PSUM") as pso:
            # --- loads: slab split as single rows up front (fast first
            # matmul) then two-row chunks round-robin.  xt/w1vr are deferred
            # to gpsimd's idle mid-kernel window so the slab and T traffic
            # own clean queues. ---
            s_weff = sb.tile([128, M2], bf16)
            nc.gpsimd.dma_start(s_weff, weff_d.ap())
            s_hsel = sb.tile([128, ROWS], bf16)
            nc.gpsimd.dma_start(s_hsel, hsel_d.ap())
            s_slab = sb.tile([128, SLAB_R, SLAB_W, S2], bf16)
            chunks = (
                (0, 1, nc.sync), (1, 2, nc.scalar), (2, 4, nc.gpsimd),
                (4, 6, nc.sync), (6, 8, nc.scalar), (8, 10, nc.gpsimd),
                (10, 12, nc.sync),
            )
            for r0, r1, e in chunks:
                e.dma_start(
                    s_slab[:, r0:r1, :, :], slab_d.ap()[:, r0:r1, :, :]
                )
            # xt/w1vr tiles are loaded later, between the T writebacks, so
            # their bulk transfers don't steal DMA-pool bandwidth from the
            # slab chunks that feed the conv matmuls
            s_xt = sb.tile([128, 8, HW], bf16)
            s_w1vr = sb.tile([128, 8, C], bf16)

            # --- phase A: T[(tap,tp), (w,t2)] = weff^T @ slab, one matmul per
            # slab row; copies transpose to t2-major (tap', t2, row, w) ---
            s_T = sb.tile([M2, S2, SLAB_R, SLAB_W], bf16)
            for row in range(SLAB_R):
                p_t = ps.tile([M2, SLAB_W, S2], f32, tag="pt")
                nc.tensor.matmul(
                    p_t, s_weff, s_slab[:, row, :, :], start=True, stop=True
                )
                dst = s_T[:, :, row, :]
                if row % 2 == 0:
                    nc.scalar.copy(dst, p_t.transpose([0, 2, 1]))
                else:
                    nc.vector.tensor_copy(dst, p_t.transpose([0, 2, 1]))

            # --- T to DRAM: 5 dy-class writes of dy-shifted 8-row windows,
            # each followed by its two (tp) 3-dim batched gathers into
            # R[(tp,t2,h), (dy,dx), w].  Write dy depends on copies for rows
            # dy..dy+7 only, so early dy classes pipeline behind the conv. ---
            s_R = sb.tile([128, NTAP, W], bf16)
            g_engs = (nc.sync, nc.gpsimd, nc.scalar, nc.sync, nc.scalar)
            for dy in range(KS):
                e = g_engs[dy]
                e.dma_start(
                    td_d.ap()[2 * KS * dy : 2 * KS * dy + 2 * KS],
                    s_T[2 * KS * dy : 2 * KS * dy + 2 * KS, :, dy : dy + ROWS, :],
                )
                for tp in range(2):
                    src = bass.AP(
                        tensor=td_d.ap().tensor,
                        offset=2 * KS * TAPW * dy + TAPW * tp,
                        ap=[[SLAB_W, S2 * ROWS], [2 * TAPW + 1, KS], [1, W]],
                    )
                    e.dma_start(
                        s_R[64 * tp : 64 * tp + 64, KS * dy : KS * dy + KS, :], src
                    )
                if dy == 1:
                    # gpsimd's queue is idle from here to the softmax: stream
                    # the V-path operands on it without delaying the
                    # dy2..dy4 writes/gathers on sync/scalar
                    nc.gpsimd.dma_start(s_w1vr, w1vr_d.ap())
                    nc.gpsimd.dma_start(s_xt[:, 0:4, :], xt_d.ap()[:, 0:4, :])
                    nc.gpsimd.dma_start(s_xt[:, 4:8, :], xt_d.ap()[:, 4:8, :])



            # --- tap reduce as a contiguous pairwise tree (strided
            # tensor_reduce over the tap axis runs ~3.5x below DVE peak) ---
            s_t1 = sb.tile([128, 12, W], f32)
            nc.vector.tensor_tensor(
                s_t1, s_R[:, 0:12, :], s_R[:, 12:24, :], op=ALU.add
            )
            s_t2 = sb.tile([128, 6, W], f32)
            nc.vector.tensor_tensor(
                s_t2, s_t1[:, 0:6, :], s_t1[:, 6:12, :], op=ALU.add
            )
            s_t3 = sb.tile([128, 3, W], f32)
            nc.vector.tensor_tensor(
                s_t3, s_t2[:, 0:3, :], s_t2[:, 3:6, :], op=ALU.add
            )
            s_t4 = sb.tile([128, W], f32)
            nc.vector.tensor_tensor(
                s_t4, s_t3[:, 0, :], s_t3[:, 1, :], op=ALU.add
            )
            s_t5 = sb.tile([128, W], f32)
            nc.vector.tensor_tensor(s_t5, s_t3[:, 2, :], s_R[:, 24, :], op=ALU.add)
            s_sk = sb.tile([128, W], f32)  # [(tp,t2,h), w]
            nc.vector.tensor_tensor(s_sk, s_t4, s_t5, op=ALU.add)

            # --- e = exp(sK); den via indicator-matmul on PE ---
            s_e16 = sb.tile([128, W], bf16)
            nc.scalar.activation(s_e16, s_sk, AF.Exp)
            p_den = pso.tile([ROWS, W], f32, tag="den")
            nc.tensor.matmul(p_den, s_hsel, s_e16, start=True, stop=True)
            s_rcp = sb.tile([ROWS, W], f32)
            nc.vector.reciprocal(s_rcp, p_den)
            nc.gpsimd.dma_start(dn_d.ap(), s_rcp)

            # --- bounce e to frame-major [t, hw] (2 writes, one per parity);
            # replicate to [(ci8,t), hw] with two parallel 0-stride reads ---
            for tp, e in ((0, nc.scalar), (1, nc.sync)):
                e.dma_start(
                    bass.AP(
                        tensor=ed_d.ap().tensor,
                        offset=HW * tp,
                        ap=[[2 * HW, S2], [W, ROWS], [1, W]],
                    ),
                    s_e16[64 * tp : 64 * tp + 64, :],
                )
            s_eb = sb.tile([128, HW], bf16)
            for half, e in ((0, nc.scalar), (1, nc.sync)):
                e.dma_start(
                    s_eb[64 * half : 64 * half + 64, :],
                    bass.AP(
                        tensor=ed_d.ap().tensor,
                        offset=0,
                        ap=[[0, 4], [HW, S], [1, HW]],
                    ),
                )

            # --- V path: xattn = x_t * e in four chunks so the V matmuls
            # pipeline tightly behind the multiplies ---
            s_xa = sb.tile([128, 8, HW], bf16)
            ebb = s_eb.unsqueeze(1).broadcast_to((128, 2, HW))
            p_o = pso.tile([C, HW], f32, tag="out")
            for q in range(4):
                nc.vector.tensor_tensor(
                    s_xa[:, 2 * q : 2 * q + 2, :],
                    s_xt[:, 2 * q : 2 * q + 2, :],
                    ebb,
                    op=ALU.mult,
                )
            for g in range(8):
                nc.tensor.matmul(
                    p_o,
                    s_w1vr[:, g, :],
                    s_xa[:, g, :],
                    start=(g == 0),
                    stop=(g == 7),
                )
            # raw (unnormalized) output; the host divides by den via dn
            s_o = sb.tile([C, HW], f32)
            nc.vector.tensor_copy(s_o, p_o)
            nc.sync.dma_start(o_d.ap(), s_o)

    nc.compile()
    return nc


def _get_module():
    global _MODULE
    if _MODULE is None:
        _MODULE = _build_module()
    return _MODULE


def make_host_inputs(x, w1, b1, w2, b2):
    """Host-side precompute: folded weights + per-core reflect-padded slices."""
    x = np.ascontiguousarray(np.asarray(x, np.float32))
    w1 = np.asarray(w1, np.float32)
    w2 = np.asarray(w2, np.float32)

    w1K = w1[C : 2 * C, :, 0, 0]          # [c, ci]
    w2K = w2[0, C : 2 * C]                # [c, 5, 5]
    weff = np.einsum("ci,cyx->iyx", w1K, w2K).reshape(C, NTAP)
    # weff2[(ci,tp), (tap,tpo)] = weff[ci, tap] * [tpo == tp]
    weff2 = np.zeros((128, M2), np.float32)
    for tp in range(2):
        weff2[tp::2, tp::2] = weff
    weff2 = weff2.astype(BF16)
    w1V = w1[2 * C :, :, 0, 0]            # [co, ci]

    # w1vr[(ci8,t), g, co] = w1V[co, 8g+ci8]
    tmp = w1V.T.reshape(8, 8, C)                      # (g, ci8, co)
    w1vr = np.ascontiguousarray(
        np.broadcast_to(tmp[:, :, None, :], (8, 8, S, C))
        .transpose(1, 2, 0, 3)
        .reshape(128, 8, C)
    ).astype(BF16)

    # hsel[(tp,t2,h), m] = 1 if h == m
    hsel = np.zeros((128, ROWS), np.float32)
    for p in range(128):
        hsel[p, p % ROWS] = 1.0
    hsel = hsel.astype(BF16)

    in_maps = []
    for core in range(NCORES):
        b, hc = divmod(core, 4)
        h0 = ROWS * hc
        xp = np.pad(x[b], ((0, 0), (PAD, PAD), (PAD, PAD), (0, 0)), mode="reflect")
        sl = xp[:, h0 : h0 + SLAB_R, :, :]            # [ci, r, w36, t]
        slab = np.ascontiguousarray(
            sl.reshape(C, SLAB_R, SLAB_W, S2, 2)
            .transpose(0, 4, 1, 2, 3)
            .reshape(128, SLAB_R, SLAB_W, S2)
        ).astype(BF16)
        xs = x[b][:, h0 : h0 + ROWS, :, :]            # [ci, h, w, t]
        xt = np.ascontiguousarray(
            xs.reshape(8, 8, ROWS, W, S)
            .transpose(1, 4, 0, 2, 3)
            .reshape(128, 8, HW)
        ).astype(BF16)
        in_maps.append(
            {"slab": slab, "xt": xt, "weff": weff2, "w1vr": w1vr, "hsel": hsel}
        )
    return in_maps


def assemble_output(results, b1):
    b1V = np.asarray(b1, np.float32)[2 * C :]
    out = np.empty((B, C, H, W, S), np.float32)
    for core in range(NCORES):
        b, hc = divmod(core, 4)
        h0 = ROWS * hc
        r = results[core]
        o = r["o"].reshape(C, ROWS, W) * r["dn"].reshape(1, ROWS, W)
        out[b, :, h0 : h0 + ROWS, :, :] = o[:, :, :, None]
    out += b1V[None, :, None, None, None]
    return out


def kernel(x, w1, b1, w2, b2):
    from concourse.bass_utils import run_bass_kernel_spmd

    nc = _get_module()
    in_maps = make_host_inputs(x, w1, b1, w2, b2)
    res = run_bass_kernel_spmd(nc, in_maps, core_ids=list(range(NCORES)))
    return assemble_output(res.results, b1)
